# revision 3
# baseline (speedup 1.0000x reference)
"""Trainium2 Bass kernel for CrossScoreQwen3VLTextAttention.

Sharding: tensor-parallel over heads across 8 NeuronCores. Core c owns
q heads {2c, 2c+1} and kv head c (GQA group). Each core computes its
q/k/v projections, per-head RMSNorm + RoPE, attention (softmax along the
k dimension, held on SBUF partitions via a transposed-score layout),
the attention output, and its 256-column shard of the o_proj
contraction. The 8 rank-256 o_proj partials are summed on the host
(the unshard step); the [B,H] cross-score reductions come back as tiny
per-head row vectors and are finished on the host.

Everything on-device is fp32; the TensorEngine streams fp32 at full
rate on trn2 so there is no precision/perf trade.

Layout trick: all activations live transposed ([d, s] / [k, q]) so that
every matmul contracts over the partition axis with zero on-device
transposes of the big [S, S] attention matrix. Softmax normalization is
folded into the epilogue: scores are exponentiated unnormalized, the
row sums l(q) come from a [ones|tm|vm] 3-column reduction matmul, and
attn_out is rescaled by 1/l(q) once at the end.
"""

import sys

sys.path.insert(0, "/opt/trn_rl_repo")

import numpy as np

import concourse.bass as bass
import concourse.mybir as mybir
import concourse.tile as tile
from concourse import bacc
from concourse.bass_utils import run_bass_kernel_spmd

F32 = mybir.dt.float32
AF = mybir.ActivationFunctionType

S = 2048
HID = 2048
NH = 16
NKV = 8
D = 128
NCORES = 8
HPC = NH // NCORES  # q heads per core = 2
SCALE = float(D) ** -0.5
EPS = 1e-6
NEG = -1e9 / SCALE  # raw-mask value pre-divided by scale
P = 128
NC_CHUNK = 512  # fp32 moving-operand max
N_HID_T = HID // P  # 16
N_S_T = S // P  # 16
N_CH = S // NC_CHUNK  # 4
STRIP_W = 896  # 384 + 512 diagonal strip width

_cache = {}
last_nc = None  # exposed for the test harness (TimelineSim)


def _build(mode):
    """mode: 'causal' (skip hidden tiles, on-chip strip mask),
    'zero' (no mask add), 'general' (stream full transposed mask)."""
    nc = bacc.Bacc("TRN2", target_bir_lowering=False, debug=False,
                   num_devices=NCORES)

    xT_d = nc.dram_tensor("xT", [HID, S], F32, kind="ExternalInput")
    wq_d = nc.dram_tensor("wq", [HID, HPC * D], F32, kind="ExternalInput")
    wkv_d = nc.dram_tensor("wkv", [HID, 2 * D], F32, kind="ExternalInput")
    wo_d = nc.dram_tensor("wo", [HPC * D, HID], F32, kind="ExternalInput")
    cosq_d = nc.dram_tensor("cosq", [D, S], F32, kind="ExternalInput")
    sinq_d = nc.dram_tensor("sinq", [D, S], F32, kind="ExternalInput")
    cosk_d = nc.dram_tensor("cosk", [D, S], F32, kind="ExternalInput")
    sink_d = nc.dram_tensor("sink", [D, S], F32, kind="ExternalInput")
    tmvm_d = nc.dram_tensor("tmvm", [S, 3], F32, kind="ExternalInput")
    strip_d = nc.dram_tensor("strip", [P, STRIP_W], F32, kind="ExternalInput")
    ones_col_d = nc.dram_tensor("ones_col", [P, 1], F32, kind="ExternalInput")
    ones_row_d = nc.dram_tensor("ones_row", [1, P], F32, kind="ExternalInput")
    rt_d = nc.dram_tensor("rt", [P, P], F32, kind="ExternalInput")
    ident_d = nc.dram_tensor("ident", [P, P], F32, kind="ExternalInput")
    if mode == "general":
        maskT_d = nc.dram_tensor("maskT", [S, S], F32, kind="ExternalInput")
        maskT_r = maskT_d[:].rearrange("(i p) s -> p i s", p=P)

    opart_d = nc.dram_tensor("opart", [S, HID], F32, kind="ExternalOutput")
    w3_d = nc.dram_tensor("w3", [HPC, 3, S], F32, kind="ExternalOutput")

    xT_r = xT_d[:].rearrange("(t p) s -> p t s", p=P)
    wq_r = wq_d[:].rearrange("(t p) d -> p t d", p=P)
    wkv_r = wkv_d[:].rearrange("(t p) d -> p t d", p=P)
    wo_r = wo_d[:].rearrange("(t p) n -> p t n", p=P)
    tmvm_r = tmvm_d[:].rearrange("(t p) c -> p t c", p=P)
    opart_r = opart_d[:].rearrange("(m p) n -> p m n", p=P)

    with tile.TileContext(nc) as tc:
        with (
            tc.tile_pool(name="consts", bufs=1) as cp,
            tc.tile_pool(name="work", bufs=3) as wp,
        ):
            wq_sb = cp.tile([P, N_HID_T, HPC * D], F32)
            nc.sync.dma_start(out=wq_sb[:], in_=wq_r)
            wkv_sb = cp.tile([P, N_HID_T, 2 * D], F32)
            nc.sync.dma_start(out=wkv_sb[:], in_=wkv_r)
            wo_sb = cp.tile([P, HPC, HID], F32)
            nc.sync.dma_start(out=wo_sb[:], in_=wo_r)
            tm_sb = cp.tile([P, N_S_T, 3], F32)
            nc.sync.dma_start(out=tm_sb[:], in_=tmvm_r)
            strip_sb = cp.tile([P, STRIP_W], F32)
            nc.sync.dma_start(out=strip_sb[:], in_=strip_d[:])
            ones_col = cp.tile([P, 1], F32)
            nc.sync.dma_start(out=ones_col[:], in_=ones_col_d[:])
            ones_row = cp.tile([1, P], F32)
            nc.sync.dma_start(out=ones_row[:], in_=ones_row_d[:])
            rt_sb = cp.tile([P, P], F32)
            nc.sync.dma_start(out=rt_sb[:], in_=rt_d[:])
            ident_sb = cp.tile([P, P], F32)
            nc.sync.dma_start(out=ident_sb[:], in_=ident_d[:])
            zero_b = cp.tile([P, 1], F32)
            nc.vector.memset(zero_b[:], 0.0)
            eps_b = cp.tile([1, 1], F32)
            nc.vector.memset(eps_b[:], EPS)

            qt_sb = [cp.tile([D, S], F32, name=f"qt{h}_sb") for h in range(HPC)]
            kt_sb = cp.tile([D, S], F32)
            vt_sb = cp.tile([D, S], F32)  # V^T, [d, k]
            v_sb = cp.tile([P, S], F32)  # V blocks, [k-within-tile, 16*d]
            ao_sb = [cp.tile([D, S], F32, name=f"ao{h}_sb") for h in range(HPC)]

            # ---- Phase 1: projections (transposed), RMSNorm + RoPE ----
            with tc.tile_pool(name="psum1", bufs=1, space="PSUM") as pp1:
                for j in range(N_CH):
                    js = slice(j * NC_CHUNK, (j + 1) * NC_CHUNK)
                    ps_q0 = pp1.tile([P, NC_CHUNK], F32, tag="pq0")
                    ps_q1 = pp1.tile([P, NC_CHUNK], F32, tag="pq1")
                    ps_k = pp1.tile([P, NC_CHUNK], F32, tag="pk")
                    ps_v = pp1.tile([P, NC_CHUNK], F32, tag="pv")
                    for t in range(N_HID_T):
                        xblk = wp.tile([P, NC_CHUNK], F32, tag="xblk", bufs=3)
                        nc.sync.dma_start(out=xblk[:], in_=xT_r[:, t, js])
                        fl = dict(start=(t == 0), stop=(t == N_HID_T - 1))
                        nc.tensor.matmul(ps_q0[:], wq_sb[:, t, 0:D], xblk[:], **fl)
                        nc.tensor.matmul(ps_q1[:], wq_sb[:, t, D:2 * D], xblk[:], **fl)
                        nc.tensor.matmul(ps_k[:], wkv_sb[:, t, 0:D], xblk[:], **fl)
                        nc.tensor.matmul(ps_v[:], wkv_sb[:, t, D:2 * D], xblk[:], **fl)
                    # V^T needs no norm/rope
                    nc.scalar.copy(vt_sb[:, js], ps_v[:])
                    for (ps_u, cos_d, sin_d, out_t) in (
                        (ps_q0, cosq_d, sinq_d, qt_sb[0]),
                        (ps_q1, cosq_d, sinq_d, qt_sb[1]),
                        (ps_k, cosk_d, sink_d, kt_sb),
                    ):
                        cos_t = wp.tile([D, NC_CHUNK], F32, tag="cs", bufs=4)
                        nc.sync.dma_start(out=cos_t[:], in_=cos_d[:, js])
                        sin_t = wp.tile([D, NC_CHUNK], F32, tag="cs", bufs=4)
                        nc.sync.dma_start(out=sin_t[:], in_=sin_d[:, js])
                        raw = wp.tile([P, NC_CHUNK], F32, tag="raw", bufs=2)
                        nc.vector.tensor_copy(raw[:], ps_u[:])
                        sq = wp.tile([P, NC_CHUNK], F32, tag="sq", bufs=2)
                        nc.scalar.activation(sq[:], raw[:], AF.Square, bias=zero_b[:])
                        ps_ssq = pp1.tile([1, NC_CHUNK], F32, tag="pssq")
                        nc.tensor.matmul(ps_ssq[:], ones_col[:], sq[:],
                                         start=True, stop=True)
                        # sqrt(mean_sq + eps), then 1/x on DVE (ACT rsqrt banned)
                        rms = wp.tile([1, NC_CHUNK], F32, tag="rms", bufs=2)
                        nc.scalar.activation(rms[:], ps_ssq[:], AF.Sqrt,
                                             bias=eps_b[:], scale=1.0 / D)
                        rinv = wp.tile([1, NC_CHUNK], F32, tag="rinv", bufs=2)
                        nc.vector.reciprocal(rinv[:], rms[:])
                        ps_rb = pp1.tile([P, NC_CHUNK], F32, tag="prb")
                        nc.tensor.matmul(ps_rb[:], ones_row[:], rinv[:],
                                         start=True, stop=True)
                        un = wp.tile([P, NC_CHUNK], F32, tag="un", bufs=2)
                        nc.vector.tensor_mul(un[:], raw[:], ps_rb[:])
                        ps_rot = pp1.tile([P, NC_CHUNK], F32, tag="prot")
                        nc.tensor.matmul(ps_rot[:], rt_sb[:], un[:],
                                         start=True, stop=True)
                        t1 = wp.tile([P, NC_CHUNK], F32, tag="t1", bufs=2)
                        nc.vector.tensor_mul(t1[:], un[:], cos_t[:])
                        t2 = wp.tile([P, NC_CHUNK], F32, tag="t2", bufs=2)
                        nc.vector.tensor_mul(t2[:], ps_rot[:], sin_t[:])
                        nc.vector.tensor_add(out_t[:, js], t1[:], t2[:])

            # ---- Phase 2: V transpose + attention per head ----
            with tc.tile_pool(name="psum2", bufs=1, space="PSUM") as pp2:
                for i in range(N_S_T):
                    ds = slice(i * P, (i + 1) * P)
                    ps_tp = pp2.tile([P, P], F32, tag="ptp", bufs=2)
                    nc.tensor.transpose(ps_tp[:], vt_sb[:, ds], ident_sb[:])
                    nc.scalar.copy(v_sb[:, ds], ps_tp[:])

                for h in range(HPC):
                    for j in range(N_CH):
                        js = slice(j * NC_CHUNK, (j + 1) * NC_CHUNK)
                        nvis = 4 * j + 4 if mode == "causal" else N_S_T
                        ps_o = pp2.tile([P, NC_CHUNK], F32, tag="po")
                        ps_c = pp2.tile([3, NC_CHUNK], F32, tag="pc")
                        for i in range(nvis):
                            ks = slice(i * P, (i + 1) * P)
                            ps_s = pp2.tile([P, NC_CHUNK], F32, tag="ps", bufs=2)
                            nc.tensor.matmul(ps_s[:], kt_sb[:, ks],
                                             qt_sb[h][:, js],
                                             start=True, stop=True)
                            e = wp.tile([P, NC_CHUNK], F32, tag="e", bufs=3)
                            delta = NC_CHUNK * j - P * i
                            if mode == "causal" and delta <= 126:
                                off = 384 + delta
                                sm = wp.tile([P, NC_CHUNK], F32, tag="sm", bufs=2)
                                nc.vector.tensor_add(
                                    sm[:], ps_s[:],
                                    strip_sb[:, off:off + NC_CHUNK])
                                nc.scalar.activation(e[:], sm[:], AF.Exp,
                                                     bias=zero_b[:], scale=SCALE)
                            elif mode == "general":
                                mb = wp.tile([P, NC_CHUNK], F32, tag="mb", bufs=4)
                                nc.sync.dma_start(out=mb[:],
                                                  in_=maskT_r[:, i, js])
                                sm = wp.tile([P, NC_CHUNK], F32, tag="sm", bufs=2)
                                nc.vector.tensor_add(sm[:], ps_s[:], mb[:])
                                nc.scalar.activation(e[:], sm[:], AF.Exp,
                                                     bias=zero_b[:], scale=SCALE)
                            else:
                                nc.scalar.activation(e[:], ps_s[:], AF.Exp,
                                                     bias=zero_b[:], scale=SCALE)
                            fl = dict(start=(i == 0), stop=(i == nvis - 1))
                            nc.tensor.matmul(ps_c[:], tm_sb[:, i, :], e[:], **fl)
                            nc.tensor.matmul(ps_o[:], v_sb[:, ks], e[:], **fl)
                        rcp = wp.tile([1, NC_CHUNK], F32, tag="rcp", bufs=2)
                        nc.vector.reciprocal(rcp[:], ps_c[0:1, :])
                        ps_rb2 = pp2.tile([P, NC_CHUNK], F32, tag="prb2")
                        nc.tensor.matmul(ps_rb2[:], ones_row[:], rcp[:],
                                         start=True, stop=True)
                        rbs = wp.tile([P, NC_CHUNK], F32, tag="rbs", bufs=2)
                        nc.scalar.copy(rbs[:], ps_rb2[:])
                        nc.vector.tensor_mul(ao_sb[h][:, js], ps_o[:], rbs[:])
                        w3b = wp.tile([3, NC_CHUNK], F32, tag="w3b", bufs=2)
                        nc.scalar.copy(w3b[:], ps_c[:])
                        nc.sync.dma_start(out=w3_d[h][:, js], in_=w3b[:])

            # ---- Phase 3: o_proj partial: out[s, n] over this core's 256 d' ----
            with tc.tile_pool(name="psum3", bufs=1, space="PSUM") as pp3:
                for m in range(N_S_T):
                    ms = slice(m * P, (m + 1) * P)
                    for n in range(N_CH):
                        ns = slice(n * NC_CHUNK, (n + 1) * NC_CHUNK)
                        ps_op = pp3.tile([P, NC_CHUNK], F32, tag="pop", bufs=2)
                        for h in range(HPC):
                            nc.tensor.matmul(ps_op[:], ao_sb[h][:, ms],
                                             wo_sb[:, h, ns],
                                             start=(h == 0), stop=(h == HPC - 1))
                        ob = wp.tile([P, NC_CHUNK], F32, tag="ob", bufs=2)
                        if (m * N_CH + n) % 2 == 0:
                            nc.vector.tensor_copy(ob[:], ps_op[:])
                        else:
                            nc.scalar.copy(ob[:], ps_op[:])
                        nc.sync.dma_start(out=opart_r[:, m, ns], in_=ob[:])


    nc.compile()
    return nc


def _detect_mode(mask):
    if not np.any(mask):
        return "zero"
    tri = np.triu(np.ones((S, S), np.bool_), k=1)
    causal = np.where(tri, np.float32(-1e9), np.float32(0.0))
    if np.array_equal(mask, causal):
        return "causal"
    return "general"


def kernel(hidden_states, wq, wk, wv, wo, q_norm_w, k_norm_w, cos, sin,
           attention_mask, vision_mask, text_mask):
    global last_nc
    f32 = np.float32
    x = np.ascontiguousarray(np.asarray(hidden_states, f32)[0])  # [S, HID]
    xT = np.ascontiguousarray(x.T)
    wq = np.asarray(wq, f32)
    wk = np.asarray(wk, f32)
    wv = np.asarray(wv, f32)
    wo = np.asarray(wo, f32)
    qw = np.asarray(q_norm_w, f32)
    kw = np.asarray(k_norm_w, f32)
    cos2 = np.asarray(cos, f32)[0]  # [S, D]
    sin2 = np.asarray(sin, f32)[0]
    mask = np.ascontiguousarray(np.asarray(attention_mask, f32)[0, 0])
    vm = np.asarray(vision_mask)[0].astype(f32)
    tm = np.asarray(text_mask)[0].astype(f32)

    mode = _detect_mode(mask)
    if mode not in _cache:
        _cache[mode] = _build(mode)
    nc = _cache[mode]
    last_nc = nc

    # Fold the per-dim norm weights into the RoPE tables:
    # q_rope = qn*w*cos + rot(qn)*w_rot*sin, w_rot[j] = w[(j+64)%128]
    qw_rot = np.roll(qw, -(D // 2))
    kw_rot = np.roll(kw, -(D // 2))
    cosq = np.ascontiguousarray((cos2 * qw).T)
    sinq = np.ascontiguousarray((sin2 * qw_rot).T)
    cosk = np.ascontiguousarray((cos2 * kw).T)
    sink = np.ascontiguousarray((sin2 * kw_rot).T)

    tmvm = np.stack([np.ones(S, f32), tm, vm], axis=1)
    tmvm = np.ascontiguousarray(tmvm)
    pi = np.arange(P, dtype=np.int64)
    ui = np.arange(STRIP_W, dtype=np.int64)
    strip = np.where(pi[:, None] <= ui[None, :] - 384,
                     f32(0.0), f32(NEG)).astype(f32)
    ones_col = np.ones((P, 1), f32)
    ones_row = np.ones((1, P), f32)
    rt = np.zeros((P, P), f32)
    rt[np.arange(64) + 64, np.arange(64)] = -1.0
    rt[np.arange(64), np.arange(64) + 64] = 1.0
    ident = np.eye(P, dtype=f32)

    in_maps = []
    for c in range(NCORES):
        im = {
            "xT": xT,
            "wq": np.ascontiguousarray(wq[:, c * HPC * D:(c + 1) * HPC * D]),
            "wkv": np.ascontiguousarray(
                np.concatenate([wk[:, c * D:(c + 1) * D],
                                wv[:, c * D:(c + 1) * D]], axis=1)),
            "wo": np.ascontiguousarray(wo[c * HPC * D:(c + 1) * HPC * D, :]),
            "cosq": cosq, "sinq": sinq, "cosk": cosk, "sink": sink,
            "tmvm": tmvm, "strip": strip,
            "ones_col": ones_col, "ones_row": ones_row,
            "rt": rt, "ident": ident,
        }
        if mode == "general":
            im["maskT"] = np.ascontiguousarray(mask.T / f32(SCALE))
        in_maps.append(im)

    res = run_bass_kernel_spmd(nc, in_maps, list(range(NCORES)))

    out = np.zeros((S, HID), f32)
    for c in range(NCORES):
        out += res.results[c]["opart"]
    out = out.reshape(1, S, HID)

    nv = vm.sum()
    nt = tm.sum()
    cross = np.zeros(NH, f32)
    for c in range(NCORES):
        w3 = res.results[c]["w3"]  # [HPC, 3, S]
        for h in range(HPC):
            l = w3[h, 0]
            wt = w3[h, 1] / l
            wv_ = w3[h, 2] / l
            v2t = float(vm @ wt) / max(nv, 1.0)
            t2v = float(tm @ wv_) / max(nt, 1.0)
            valid = 1.0 if (nv > 0 and nt > 0) else 0.0
            cross[c * HPC + h] = (v2t + t2v) * valid / max(valid, 1.0)

    return out, cross


# revision 6
# speedup vs baseline: 2.5992x; 2.5992x over previous
"""Trainium2 Bass kernel for CrossScoreQwen3VLTextAttention.

Sharding: tensor-parallel over heads across 8 NeuronCores. Core c owns
q heads {2c, 2c+1} and kv head c (GQA group). Each core computes its
q/k/v projections, per-head RMSNorm + RoPE, attention (softmax along the
k dimension, held on SBUF partitions via a transposed-score layout),
the attention output, and its 256-column shard of the o_proj
contraction. The 8 rank-256 o_proj partials are summed on the host
(the unshard step); the [B,H] cross-score reductions come back as tiny
per-head row vectors and are finished on the host.

Layout trick: all activations live transposed ([d, s] / [k, q]) so that
every matmul contracts over the partition axis with zero on-device
transposes of the big [S, S] attention matrix. Softmax normalization is
folded into the epilogue: scores are exponentiated unnormalized, the
row sums l(q) come from a [ones|tm|vm] 3-column reduction matmul, and
attn_out is rescaled by 1/l(q) once at the end.

o_proj is interleaved with attention per 512-wide q-chunk so PE never
drains, and all PSUM lives in one pool (7 banks, manual tag rings) so
no phase transition serializes on PSUM reallocation.
"""

import sys

sys.path.insert(0, "/opt/trn_rl_repo")

import numpy as np

import concourse.bass as bass
import concourse.mybir as mybir
import concourse.tile as tile
from concourse import bacc
from concourse.bass_utils import run_bass_kernel_spmd

F32 = mybir.dt.float32
# PE runs fp32 at 1/4 rate; float32r (same bits, reduced multiply
# precision) streams at full rate for moving dim >= 256, so every
# matmul-facing tensor is typed float32r.
F32R = mybir.dt.float32r
AF = mybir.ActivationFunctionType

S = 2048
HID = 2048
NH = 16
NKV = 8
D = 128
NCORES = 8
HPC = NH // NCORES  # q heads per core = 2
SCALE = float(D) ** -0.5
EPS = 1e-6
NEG = -1e9 / SCALE  # raw-mask value pre-divided by scale
P = 128
CH = 512  # fp32 moving-operand max
N_HID_T = HID // P  # 16
N_S_T = S // P  # 16
N_CH = S // CH  # 4
STRIP_W = 896  # 384 + 512 diagonal strip width

_cache = {}
last_nc = None  # exposed for the test harness (TimelineSim)


def _build(mode):
    """mode: 'causal' (skip hidden tiles, on-chip strip mask),
    'zero' (no mask add), 'general' (stream full transposed mask)."""
    nc = bacc.Bacc("TRN2", target_bir_lowering=False, debug=False,
                   num_devices=NCORES)

    xT_d = nc.dram_tensor("xT", [HID, S], F32R, kind="ExternalInput")
    wq_d = nc.dram_tensor("wq", [HID, HPC * D], F32R, kind="ExternalInput")
    wkv_d = nc.dram_tensor("wkv", [HID, 2 * D], F32R, kind="ExternalInput")
    wo_d = nc.dram_tensor("wo", [HPC * D, HID], F32R, kind="ExternalInput")
    cosq_d = nc.dram_tensor("cosq", [D, S], F32, kind="ExternalInput")
    sinq_d = nc.dram_tensor("sinq", [D, S], F32, kind="ExternalInput")
    cosk_d = nc.dram_tensor("cosk", [D, S], F32, kind="ExternalInput")
    sink_d = nc.dram_tensor("sink", [D, S], F32, kind="ExternalInput")
    tmvm_d = nc.dram_tensor("tmvm", [S, 3], F32R, kind="ExternalInput")
    strip_d = nc.dram_tensor("strip", [P, STRIP_W], F32, kind="ExternalInput")
    ones_col_d = nc.dram_tensor("ones_col", [P, 1], F32R, kind="ExternalInput")
    ones_row_d = nc.dram_tensor("ones_row", [1, P], F32R, kind="ExternalInput")
    rt_d = nc.dram_tensor("rt", [P, P], F32R, kind="ExternalInput")
    ident_d = nc.dram_tensor("ident", [P, P], F32, kind="ExternalInput")
    if mode == "general":
        maskT_d = nc.dram_tensor("maskT", [S, S], F32, kind="ExternalInput")
        maskT_r = maskT_d[:].rearrange("(i p) s -> p i s", p=P)

    opart_d = nc.dram_tensor("opart", [S, HID], F32, kind="ExternalOutput")
    w3_d = nc.dram_tensor("w3", [HPC, 3, S], F32, kind="ExternalOutput")

    xT_r = xT_d[:].rearrange("(t p) s -> p t s", p=P)
    wq_r = wq_d[:].rearrange("(t p) d -> p t d", p=P)
    wkv_r = wkv_d[:].rearrange("(t p) d -> p t d", p=P)
    wo_r = wo_d[:].rearrange("(t p) n -> p t n", p=P)
    tmvm_r = tmvm_d[:].rearrange("(t p) c -> p t c", p=P)
    opart_r = opart_d[:].rearrange("(m p) n -> p m n", p=P)

    with tile.TileContext(nc) as tc:
        with (
            nc.allow_low_precision(reason="float32r matmul operands"),
            tc.tile_pool(name="consts", bufs=1) as cp,
            tc.tile_pool(name="work", bufs=3) as wp,
            tc.tile_pool(name="pp", bufs=1, space="PSUM") as pp,
        ):
            # q/kv weights split per hid-tile on the sync queue so the first
            # projection matmul only waits on tile 0; everything not needed
            # until later goes through gpsimd (SWDGE, separate queue).
            wq_sb = cp.tile([P, N_HID_T, HPC * D], F32R)
            wkv_sb = cp.tile([P, N_HID_T, 2 * D], F32R)
            for t in range(N_HID_T):
                nc.sync.dma_start(out=wq_sb[:, t, :], in_=wq_r[:, t, :])
                nc.sync.dma_start(out=wkv_sb[:, t, :], in_=wkv_r[:, t, :])
            ones_col = cp.tile([P, 1], F32R)
            nc.gpsimd.dma_start(out=ones_col[:], in_=ones_col_d[:])
            ones_row = cp.tile([1, P], F32R)
            nc.gpsimd.dma_start(out=ones_row[:], in_=ones_row_d[:])
            rt_sb = cp.tile([P, P], F32R)
            nc.gpsimd.dma_start(out=rt_sb[:], in_=rt_d[:])
            ident_sb = cp.tile([P, P], F32)
            nc.gpsimd.dma_start(out=ident_sb[:], in_=ident_d[:])
            strip_sb = cp.tile([P, STRIP_W], F32)
            nc.gpsimd.dma_start(out=strip_sb[:], in_=strip_d[:])
            tm_sb = cp.tile([P, N_S_T, 3], F32R)
            nc.gpsimd.dma_start(out=tm_sb[:], in_=tmvm_r)
            wo_sb = cp.tile([P, HPC, HID], F32R)
            nc.gpsimd.dma_start(out=wo_sb[:], in_=wo_r)
            zero_b = cp.tile([P, 1], F32)
            nc.vector.memset(zero_b[:], 0.0)
            eps_b = cp.tile([1, 1], F32)
            nc.vector.memset(eps_b[:], EPS)

            qt_sb = [cp.tile([D, S], F32R, name=f"qt{h}_sb") for h in range(HPC)]
            kt_sb = cp.tile([D, S], F32R)
            vt_sb = cp.tile([D, S], F32)  # V^T, [d, k]
            v_sb = cp.tile([P, S], F32R)  # V blocks, [k-within-tile, 16*d]
            ao_sb = [cp.tile([D, S], F32R, name=f"ao{h}_sb") for h in range(HPC)]

            # ---- Phase 1: projections (transposed), RMSNorm + RoPE ----
            for j in range(N_CH):
                js = slice(j * CH, (j + 1) * CH)
                ps_q0 = pp.tile([P, CH], F32, tag="pA")
                ps_q1 = pp.tile([P, CH], F32, tag="pB")
                ps_k = pp.tile([P, CH], F32, tag="pC")
                ps_v = pp.tile([P, CH], F32, tag="pD")
                for t in range(N_HID_T):
                    xblk = wp.tile([P, CH], F32R, tag="xblk", bufs=4)
                    nc.sync.dma_start(out=xblk[:], in_=xT_r[:, t, js])
                    fl = dict(start=(t == 0), stop=(t == N_HID_T - 1))
                    nc.tensor.matmul(ps_q0[:], wq_sb[:, t, 0:D], xblk[:], **fl)
                    nc.tensor.matmul(ps_q1[:], wq_sb[:, t, D:2 * D], xblk[:], **fl)
                    nc.tensor.matmul(ps_k[:], wkv_sb[:, t, 0:D], xblk[:], **fl)
                    nc.tensor.matmul(ps_v[:], wkv_sb[:, t, D:2 * D], xblk[:], **fl)
                # V^T needs no norm/rope
                nc.scalar.copy(vt_sb[:, js], ps_v[:])
                for (ps_u, cos_d, sin_d, out_t) in (
                    (ps_q0, cosq_d, sinq_d, qt_sb[0]),
                    (ps_q1, cosq_d, sinq_d, qt_sb[1]),
                    (ps_k, cosk_d, sink_d, kt_sb),
                ):
                    cos_t = wp.tile([D, CH], F32, tag="cs", bufs=4)
                    nc.sync.dma_start(out=cos_t[:], in_=cos_d[:, js])
                    sin_t = wp.tile([D, CH], F32, tag="cs", bufs=4)
                    nc.sync.dma_start(out=sin_t[:], in_=sin_d[:, js])
                    raw = wp.tile([P, CH], F32, tag="raw", bufs=2)
                    nc.vector.tensor_copy(raw[:], ps_u[:])
                    sq = wp.tile([P, CH], F32R, tag="sq", bufs=2)
                    nc.scalar.activation(sq[:], raw[:], AF.Square, bias=zero_b[:])
                    ps_ssq = pp.tile([1, CH], F32, tag="pE")
                    nc.tensor.matmul(ps_ssq[:], ones_col[:], sq[:],
                                     start=True, stop=True)
                    # sqrt(mean_sq + eps), then 1/x on DVE (ACT rsqrt banned)
                    rms = wp.tile([1, CH], F32, tag="rms", bufs=2)
                    nc.scalar.activation(rms[:], ps_ssq[:], AF.Sqrt,
                                         bias=eps_b[:], scale=1.0 / D)
                    rinv = wp.tile([1, CH], F32R, tag="rinv", bufs=2)
                    nc.vector.reciprocal(rinv[:], rms[:])
                    ps_rb = pp.tile([P, CH], F32, tag="pF")
                    nc.tensor.matmul(ps_rb[:], ones_row[:], rinv[:],
                                     start=True, stop=True)
                    un = wp.tile([P, CH], F32R, tag="un", bufs=2)
                    nc.vector.tensor_mul(un[:], raw[:], ps_rb[:])
                    ps_rot = pp.tile([P, CH], F32, tag="pF")
                    nc.tensor.matmul(ps_rot[:], rt_sb[:], un[:],
                                     start=True, stop=True)
                    t1 = wp.tile([P, CH], F32, tag="t1", bufs=2)
                    nc.vector.tensor_mul(t1[:], un[:], cos_t[:])
                    t2 = wp.tile([P, CH], F32, tag="t2", bufs=2)
                    nc.vector.tensor_mul(t2[:], ps_rot[:], sin_t[:])
                    nc.vector.tensor_add(out_t[:, js], t1[:], t2[:])
                # V transpose for the 4 k-tiles of this chunk
                for i in range(4 * j, 4 * j + 4):
                    dsl = slice(i * P, (i + 1) * P)
                    ps_tp = pp.tile([P, P], F32, tag="pG")
                    nc.tensor.transpose(ps_tp[:], vt_sb[:, dsl], ident_sb[:])
                    nc.vector.tensor_copy(v_sb[:, dsl], ps_tp[:])

            # ---- Phase 2+3 fused per q-chunk: attention then o_proj ----
            for j in range(N_CH):
                js = slice(j * CH, (j + 1) * CH)
                nvis = 4 * j + 4 if mode == "causal" else N_S_T
                for h in range(HPC):
                    ps_o = pp.tile([P, CH], F32, tag="pC")
                    ps_c = pp.tile([3, CH], F32, tag="pE")
                    for i in range(nvis):
                        ks = slice(i * P, (i + 1) * P)
                        ps_s = pp.tile([P, CH], F32, tag=f"p{'AB'[i % 2]}")
                        nc.tensor.matmul(ps_s[:], kt_sb[:, ks],
                                         qt_sb[h][:, js],
                                         start=True, stop=True)
                        e = wp.tile([P, CH], F32R, tag="e", bufs=4)
                        delta = CH * j - P * i
                        if mode == "causal" and delta <= 126:
                            off = 384 + delta
                            sm = wp.tile([P, CH], F32, tag="sm", bufs=2)
                            nc.vector.tensor_add(
                                sm[:], ps_s[:],
                                strip_sb[:, off:off + CH])
                            nc.scalar.activation(e[:], sm[:], AF.Exp,
                                                 bias=zero_b[:], scale=SCALE)
                        elif mode == "general":
                            mb = wp.tile([P, CH], F32, tag="mb", bufs=4)
                            nc.sync.dma_start(out=mb[:], in_=maskT_r[:, i, js])
                            sm = wp.tile([P, CH], F32, tag="sm", bufs=2)
                            nc.vector.tensor_add(sm[:], ps_s[:], mb[:])
                            nc.scalar.activation(e[:], sm[:], AF.Exp,
                                                 bias=zero_b[:], scale=SCALE)
                        else:
                            nc.scalar.activation(e[:], ps_s[:], AF.Exp,
                                                 bias=zero_b[:], scale=SCALE)
                        fl = dict(start=(i == 0), stop=(i == nvis - 1))
                        nc.tensor.matmul(ps_c[:], tm_sb[:, i, :], e[:], **fl)
                        nc.tensor.matmul(ps_o[:], v_sb[:, ks], e[:], **fl)
                    rcp = wp.tile([1, CH], F32R, tag="rcp", bufs=2)
                    nc.vector.reciprocal(rcp[:], ps_c[0:1, :])
                    ps_rb2 = pp.tile([P, CH], F32, tag="pF")
                    nc.tensor.matmul(ps_rb2[:], ones_row[:], rcp[:],
                                     start=True, stop=True)
                    rbs = wp.tile([P, CH], F32, tag="rbs", bufs=2)
                    nc.scalar.copy(rbs[:], ps_rb2[:])
                    nc.vector.tensor_mul(ao_sb[h][:, js], ps_o[:], rbs[:])
                    w3b = wp.tile([3, CH], F32, tag="w3b", bufs=2)
                    nc.scalar.copy(w3b[:], ps_c[:])
                    nc.sync.dma_start(out=w3_d[h][:, js], in_=w3b[:])
                # o_proj rows for the 4 s-tiles whose attention just finished
                for m in range(4 * j, 4 * j + 4):
                    msl = slice(m * P, (m + 1) * P)
                    for n in range(N_CH):
                        ns = slice(n * CH, (n + 1) * CH)
                        ps_op = pp.tile([P, CH], F32, tag=f"p{'DG'[n % 2]}")
                        for h in range(HPC):
                            nc.tensor.matmul(ps_op[:], ao_sb[h][:, msl],
                                             wo_sb[:, h, ns],
                                             start=(h == 0), stop=(h == HPC - 1))
                        ob = wp.tile([P, CH], F32, tag="ob", bufs=4)
                        if n % 2 == 0:
                            nc.vector.tensor_copy(ob[:], ps_op[:])
                        else:
                            nc.scalar.copy(ob[:], ps_op[:])
                        nc.sync.dma_start(out=opart_r[:, m, ns], in_=ob[:])

    nc.compile()
    return nc


def _detect_mode(mask):
    if not np.any(mask):
        return "zero"
    tri = np.triu(np.ones((S, S), np.bool_), k=1)
    causal = np.where(tri, np.float32(-1e9), np.float32(0.0))
    if np.array_equal(mask, causal):
        return "causal"
    return "general"


def kernel(hidden_states, wq, wk, wv, wo, q_norm_w, k_norm_w, cos, sin,
           attention_mask, vision_mask, text_mask):
    global last_nc
    f32 = np.float32
    x = np.ascontiguousarray(np.asarray(hidden_states, f32)[0])  # [S, HID]
    xT = np.ascontiguousarray(x.T)
    wq = np.asarray(wq, f32)
    wk = np.asarray(wk, f32)
    wv = np.asarray(wv, f32)
    wo = np.asarray(wo, f32)
    qw = np.asarray(q_norm_w, f32)
    kw = np.asarray(k_norm_w, f32)
    cos2 = np.asarray(cos, f32)[0]  # [S, D]
    sin2 = np.asarray(sin, f32)[0]
    mask = np.ascontiguousarray(np.asarray(attention_mask, f32)[0, 0])
    vm = np.asarray(vision_mask)[0].astype(f32)
    tm = np.asarray(text_mask)[0].astype(f32)

    mode = _detect_mode(mask)
    if mode not in _cache:
        _cache[mode] = _build(mode)
    nc = _cache[mode]
    last_nc = nc

    # Fold the per-dim norm weights into the RoPE tables:
    # q_rope = qn*w*cos + rot(qn)*w_rot*sin, w_rot[j] = w[(j+64)%128]
    qw_rot = np.roll(qw, -(D // 2))
    kw_rot = np.roll(kw, -(D // 2))
    cosq = np.ascontiguousarray((cos2 * qw).T)
    sinq = np.ascontiguousarray((sin2 * qw_rot).T)
    cosk = np.ascontiguousarray((cos2 * kw).T)
    sink = np.ascontiguousarray((sin2 * kw_rot).T)

    tmvm = np.stack([np.ones(S, f32), tm, vm], axis=1)
    tmvm = np.ascontiguousarray(tmvm)
    pi = np.arange(P, dtype=np.int64)
    ui = np.arange(STRIP_W, dtype=np.int64)
    strip = np.where(pi[:, None] <= ui[None, :] - 384,
                     f32(0.0), f32(NEG)).astype(f32)
    ones_col = np.ones((P, 1), f32)
    ones_row = np.ones((1, P), f32)
    rt = np.zeros((P, P), f32)
    rt[np.arange(64) + 64, np.arange(64)] = -1.0
    rt[np.arange(64), np.arange(64) + 64] = 1.0
    ident = np.eye(P, dtype=f32)

    in_maps = []
    for c in range(NCORES):
        im = {
            "xT": xT,
            "wq": np.ascontiguousarray(wq[:, c * HPC * D:(c + 1) * HPC * D]),
            "wkv": np.ascontiguousarray(
                np.concatenate([wk[:, c * D:(c + 1) * D],
                                wv[:, c * D:(c + 1) * D]], axis=1)),
            "wo": np.ascontiguousarray(wo[c * HPC * D:(c + 1) * HPC * D, :]),
            "cosq": cosq, "sinq": sinq, "cosk": cosk, "sink": sink,
            "tmvm": tmvm, "strip": strip,
            "ones_col": ones_col, "ones_row": ones_row,
            "rt": rt, "ident": ident,
        }
        if mode == "general":
            im["maskT"] = np.ascontiguousarray(mask.T / f32(SCALE))
        in_maps.append(im)

    res = run_bass_kernel_spmd(nc, in_maps, list(range(NCORES)))

    out = np.zeros((S, HID), f32)
    for c in range(NCORES):
        out += res.results[c]["opart"]
    out = out.reshape(1, S, HID)

    nv = vm.sum()
    nt = tm.sum()
    cross = np.zeros(NH, f32)
    for c in range(NCORES):
        w3 = res.results[c]["w3"]  # [HPC, 3, S]
        for h in range(HPC):
            l = w3[h, 0]
            wt = w3[h, 1] / l
            wv_ = w3[h, 2] / l
            v2t = float(vm @ wt) / max(nv, 1.0)
            t2v = float(tm @ wv_) / max(nt, 1.0)
            valid = 1.0 if (nv > 0 and nt > 0) else 0.0
            cross[c * HPC + h] = (v2t + t2v) * valid / max(valid, 1.0)

    return out, cross
